# revision 13
# baseline (speedup 1.0000x reference)
"""CBOW negative-sampling loss kernel for 8 Trainium2 NeuronCores.

Math (faithful to the reference, including its [B]+[B,1] broadcast bug):
    c_b   = mean_w ctx_w[context[b, w]]               # [D]
    pos_b = log_sigmoid(emb_w[target[b]] . c_b)
    neg_b = sum_k log_sigmoid(emb_w[noise[b, k]] . c_b)
    out   = -(mean_b pos_b + mean_b neg_b) = -(sum_b (pos_b + neg_b)) / B

Strategy: shard B across the 8 cores (2048 samples each); tables cast to bf16
on the host (dots are ~1e-4 so bf16 is far inside the fp32 tolerance).
Per core, 16 blocks of 128 samples (partition = sample-in-block) are covered
by a few supergroups. Per supergroup two indirect gathers run on the Pool
SWDGE queue:
  - ctx rows land [word u][block b][D]: slot (u, b) holds word u of block b's
    128 samples, one row per partition. Context pooling is then 10
    PSUM-accumulating identity matmuls (lhsT is a constant identity, loaded
    once - no per-matmul LDWEIGHTS, no pooling matrix); the scalar engine
    downcasts c to bf16.
  - target+noise rows land [block][tgt, noise*10][D] along the partition's
    free dim.
Dots run on DVE: one bf16 multiply against broadcast c (2x mode), a chain of
2x tensor_tensor folds (128->64->32->16) replacing most of the 1x-only
tensor_reduce, then a short reduce to fp32. One tail Sigmoid(x/10) + Ln pass
on the scalar engine (Ln's accum_out) yields per-partition summed
log-sigmoid. The host sums the per-core partials and scales by -1/B.
"""

import numpy as np

V, D = 100000, 128
B, W, K = 16384, 10, 10
NCORES = 8
P = 128
B_LOCAL = B // NCORES  # 2048
NBLK = B_LOCAL // P  # 16 blocks of 128 samples
KP1 = K + 1  # 11 emb rows per sample
# Supergroup sizes in blocks: small first groups prime the compute pipeline.
SGS = [2, 2, 4, 4, 4]
assert sum(SGS) == NBLK

_LAST_RESULTS = None  # test harness introspection (exec_time_ns etc.)


def _build_bass(vocab, debug_dots=False):
    import concourse.bass as bass
    import concourse.tile as tile
    from concourse import bacc, mybir

    total_cols = NBLK * W + NBLK * KP1  # 336
    nc = bacc.Bacc(None, target_bir_lowering=False)
    dots_d = (
        nc.declare_dram_parameter(
            "dots", [P, NBLK * KP1], mybir.dt.float32, isOutput=True
        )
        if debug_dots
        else None
    )
    idx_d = nc.declare_dram_parameter(
        "idx", [P, total_cols], mybir.dt.int32, isOutput=False
    )
    ident_d = nc.declare_dram_parameter(
        "ident", [P, P], mybir.dt.bfloat16, isOutput=False
    )
    ctx_w_d = nc.declare_dram_parameter(
        "ctx_w", [vocab, D], mybir.dt.bfloat16, isOutput=False
    )
    emb_w_d = nc.declare_dram_parameter(
        "emb_w", [vocab, D], mybir.dt.bfloat16, isOutput=False
    )
    out_d = nc.declare_dram_parameter("out", [P, 1], mybir.dt.float32, isOutput=True)

    with tile.TileContext(nc) as tc:
        with (
            tc.tile_pool(name="const", bufs=1) as cpool,
            tc.tile_pool(name="work", bufs=2) as wpool,
            tc.tile_pool(name="psum", bufs=2, space="PSUM") as ppool,
        ):
            idx_sb = cpool.tile([P, total_cols], mybir.dt.int32)
            nc.sync.dma_start(out=idx_sb[:], in_=idx_d[:])
            ident_sb = cpool.tile([P, P], mybir.dt.bfloat16)
            nc.sync.dma_start(out=ident_sb[:], in_=ident_d[:])
            all_dots = cpool.tile([P, NBLK * KP1], mybir.dt.float32)
            acc = cpool.tile([P, 1], mybir.dt.float32)

            # Per-supergroup persistent tiles (no recycling -> no false deps).
            c_sbs, Tctxs, Tembs = [], [], []
            for i, nb in enumerate(SGS):
                c_sb = cpool.tile([P, nb * D], mybir.dt.bfloat16, tag=f"c{i}")
                c_sbs.append(c_sb)
                Tctx = cpool.tile([P, W * nb * D], mybir.dt.bfloat16, tag=f"C{i}")
                Tctxs.append(Tctx)
                Temb = cpool.tile([P, nb * KP1 * D], mybir.dt.bfloat16, tag=f"T{i}")
                Tembs.append(Temb)

            # All gathers up front; the SWDGE queue drains them back to back.
            col = 0
            for i, nb in enumerate(SGS):
                ctx_cols = W * nb
                emb_cols = KP1 * nb
                # ctx: col (u, b) = word u of block b's samples, one row per
                # partition (partition = sample-in-block).
                nc.gpsimd.indirect_dma_start(
                    out=Tctxs[i][:],
                    out_offset=None,
                    in_=ctx_w_d[:],
                    in_offset=bass.IndirectOffsetOnAxis(
                        ap=idx_sb[:, col : col + ctx_cols], axis=0
                    ),
                )
                col += ctx_cols
                # emb: col (b, j) = [tgt, noise*10] per sample, block-major.
                nc.gpsimd.indirect_dma_start(
                    out=Tembs[i][:],
                    out_offset=None,
                    in_=emb_w_d[:],
                    in_offset=bass.IndirectOffsetOnAxis(
                        ap=idx_sb[:, col : col + emb_cols], axis=0
                    ),
                )
                col += emb_cols

            # Context pooling: 10 PSUM-accumulating identity matmuls per
            # supergroup (c[s, :] = sum_u Tctx[s, slot u]); constant lhsT.
            for i, nb in enumerate(SGS):
                c_ps = ppool.tile([P, nb * D], mybir.dt.float32, tag="cps")
                for u in range(W):
                    nc.tensor.matmul(
                        c_ps[:],
                        lhsT=ident_sb[:],
                        rhs=Tctxs[i][:, u * nb * D : (u + 1) * nb * D],
                        start=(u == 0),
                        stop=(u == W - 1),
                    )
                nc.scalar.activation(
                    out=c_sbs[i][:],
                    in_=c_ps[:],
                    func=mybir.ActivationFunctionType.Copy,
                )

            # Dots per supergroup: multiply (2x) + fold chain (2x) + reduce.
            doff = 0
            for i, nb in enumerate(SGS):
                seg = nb * KP1
                cview = c_sbs[i][:].rearrange("p (b d) -> p b d", b=nb)
                prod = wpool.tile([P, seg * D], mybir.dt.bfloat16, tag="prod")
                nc.vector.tensor_tensor(
                    out=prod[:],
                    in0=Tembs[i][:],
                    in1=cview.unsqueeze(2).broadcast_to([P, nb, KP1, D]),
                    op=mybir.AluOpType.mult,
                )
                f64 = wpool.tile([P, seg * 64], mybir.dt.bfloat16, tag="f64")
                pv = prod[:].rearrange("p (s d) -> p s d", d=D)
                nc.vector.tensor_tensor(
                    out=f64[:],
                    in0=pv[:, :, 0:64],
                    in1=pv[:, :, 64:128],
                    op=mybir.AluOpType.add,
                )
                f32 = wpool.tile([P, seg * 32], mybir.dt.bfloat16, tag="f32")
                fv = f64[:].rearrange("p (s d) -> p s d", d=64)
                nc.vector.tensor_tensor(
                    out=f32[:],
                    in0=fv[:, :, 0:32],
                    in1=fv[:, :, 32:64],
                    op=mybir.AluOpType.add,
                )
                f16 = wpool.tile([P, seg * 16], mybir.dt.bfloat16, tag="f16")
                gv = f32[:].rearrange("p (s d) -> p s d", d=32)
                nc.vector.tensor_tensor(
                    out=f16[:],
                    in0=gv[:, :, 0:16],
                    in1=gv[:, :, 16:32],
                    op=mybir.AluOpType.add,
                )
                nc.vector.tensor_reduce(
                    out=all_dots[:, doff : doff + seg],
                    in_=f16[:].rearrange("p (s d) -> p s d", d=16),
                    axis=mybir.AxisListType.X,
                    op=mybir.AluOpType.add,
                )
                doff += seg

            # One tail pass: log-sigmoid of all dots (0.1 rescales the ctx sum
            # to a mean); Ln's accum_out emits per-partition sums.
            sig = cpool.tile([P, NBLK * KP1], mybir.dt.float32)
            nc.scalar.activation(
                out=sig[:],
                in_=all_dots[:],
                func=mybir.ActivationFunctionType.Sigmoid,
                scale=1.0 / W,
            )
            ls = cpool.tile([P, NBLK * KP1], mybir.dt.float32)
            nc.scalar.activation(
                out=ls[:],
                in_=sig[:],
                func=mybir.ActivationFunctionType.Ln,
                accum_out=acc[:, 0:1],
            )

            nc.sync.dma_start(out=out_d[:], in_=acc[:])
            if dots_d is not None:
                nc.sync.dma_start(out=dots_d[:], in_=all_dots[:])
    nc.compile()
    return nc


def _pack_indices(context, target, noise):
    """Per-core [P, 336] int32 index matrices in gather layout."""
    ctx_r = np.ascontiguousarray(context, dtype=np.int32).reshape(
        NCORES, NBLK, P, W
    )
    tgt_r = np.ascontiguousarray(target, dtype=np.int32).reshape(NCORES, NBLK, P)
    noi_r = np.ascontiguousarray(noise, dtype=np.int32).reshape(NCORES, NBLK, P, K)
    idxs = []
    for n in range(NCORES):
        cols = []
        b0 = 0
        for nb in SGS:
            # ctx cols u-major: col (u, b) partition p = context[block b0+b,
            # sample p, word u]
            csg = ctx_r[n, b0 : b0 + nb]  # [nb, P, W]
            cols.append(csg.transpose(2, 0, 1).reshape(W * nb, P).T)
            # emb cols block-major: col (b, j) = [tgt, noise] for sample p
            esg = np.concatenate(
                [tgt_r[n, b0 : b0 + nb, :, None], noi_r[n, b0 : b0 + nb]], axis=2
            )  # [nb, P, 11]
            cols.append(esg.transpose(0, 2, 1).reshape(nb * KP1, P).T)
            b0 += nb
        idxs.append(np.ascontiguousarray(np.concatenate(cols, axis=1)))
    return idxs


def kernel(context, target, noise, emb_w, ctx_w):
    global _LAST_RESULTS
    import os
    import sys

    for p in ("/root/.axon_site/_ro/trn_rl_repo", "/opt/trn_rl_repo"):
        if p not in sys.path:
            sys.path.insert(0, p)
    import ml_dtypes

    from concourse.bass_utils import run_bass_kernel_spmd

    context = np.asarray(context)
    target = np.asarray(target)
    noise = np.asarray(noise)
    bf16 = ml_dtypes.bfloat16
    emb_w = np.ascontiguousarray(np.asarray(emb_w, dtype=np.float32).astype(bf16))
    ctx_w = np.ascontiguousarray(np.asarray(ctx_w, dtype=np.float32).astype(bf16))

    debug_dots = bool(os.environ.get("KERNEL_DEBUG_DOTS"))
    nc = _build_bass(V, debug_dots=debug_dots)
    idxs = _pack_indices(context, target, noise)
    ident = np.eye(P, dtype=np.float32).astype(bf16)
    in_maps = [
        {"idx": idxs[n], "ident": ident, "ctx_w": ctx_w, "emb_w": emb_w}
        for n in range(NCORES)
    ]
    tmpdir = os.environ.get("KERNEL_TMPDIR") or None
    res = run_bass_kernel_spmd(nc, in_maps, list(range(NCORES)), tmpdir=tmpdir)
    _LAST_RESULTS = res
    if debug_dots:
        _check_dots(res, context, target, noise, emb_w, ctx_w)
    total = sum(
        float(np.sum(np.asarray(r["out"], dtype=np.float64))) for r in res.results
    )
    return np.float32(-total / B)


def _check_dots(res, context, target, noise, emb_w, ctx_w):
    """Compare on-device dot products (pre-sigmoid, x10 scale) vs numpy."""
    ctx_f = np.asarray(ctx_w, dtype=np.float32)
    emb_f = np.asarray(emb_w, dtype=np.float32)
    c_ref = ctx_f[np.asarray(context)].sum(axis=1)  # [B, D] (sum, not mean)
    eidx = np.concatenate(
        [np.asarray(target)[:, None], np.asarray(noise)], axis=1
    )  # [B, 11]
    dots_ref = np.einsum("bjd,bd->bj", emb_f[eidx], c_ref)  # [B, 11]
    worst = 0.0
    for n, r in enumerate(res.results):
        got = np.asarray(r["dots"], dtype=np.float32)  # [P, NBLK*KP1]
        b0g = n * B_LOCAL
        doff = 0
        bcum = 0
        for nb in SGS:
            for b in range(nb):
                for p_chunk in range(1):
                    ref = dots_ref[b0g + (bcum + b) * P : b0g + (bcum + b + 1) * P]
                sl = got[:, doff + b * KP1 : doff + (b + 1) * KP1]  # [P, 11]
                err = np.abs(sl - ref) / (np.abs(ref) + 1e-6)
                worst = max(worst, float(err.max()))
            doff += nb * KP1
            bcum += nb
    print(f"debug dots: worst rel err vs numpy-fp32 = {worst:.3e}")


# revision 17
# speedup vs baseline: 1.0990x; 1.0990x over previous
"""CBOW negative-sampling loss kernel for 8 Trainium2 NeuronCores.

Math (faithful to the reference, including its [B]+[B,1] broadcast bug):
    c_b   = mean_w ctx_w[context[b, w]]               # [D]
    pos_b = log_sigmoid(emb_w[target[b]] . c_b)
    neg_b = sum_k log_sigmoid(emb_w[noise[b, k]] . c_b)
    out   = -(mean_b pos_b + mean_b neg_b) = -(sum_b (pos_b + neg_b)) / B

Strategy: shard B across the 8 cores (2048 samples each); tables cast to bf16
on the host (dots are ~1e-4 so bf16 is far inside the fp32 tolerance).
Per core, 16 blocks of 128 samples (partition = sample-in-block), processed
as 9 chunks of 1-2 blocks. Per chunk two indirect gathers run on the Pool
SWDGE queue (small enough that the SWDGE descriptor ring never stalls):
  - ctx rows land [word u][block b][D] in a per-chunk tile; pooling is 10
    PSUM-accumulating identity matmuls. The identity is loaded once per chunk
    via an explicit ldweights; the matmuls set ldweights=False so the PE
    array is not reloaded (and flushed) between accumulating matmuls.
  - target+noise rows land [block][tgt, noise*10][D].
The scalar engine downcasts c to bf16 (PSUM -> SBUF). Dots run on DVE: one
bf16 multiply per chunk against broadcast c (2x mode), then per chunk-group
a chain of 2x tensor_tensor folds (128->64->32->16) and a short 1x
tensor_reduce to fp32 (the fold chain replaces most of the 1x-only
tensor_reduce). Sigmoid/Ln tables are preloaded at t=0 by dummy activations
so the tail pass (Sigmoid(x/10) + Ln with accum_out) costs ~1us. The host
sums the per-core partials and scales by -1/B.
"""

import numpy as np

V, D = 100000, 128
B, W, K = 16384, 10, 10
NCORES = 8
P = 128
B_LOCAL = B // NCORES  # 2048
NBLK = B_LOCAL // P  # 16 blocks of 128 samples
KP1 = K + 1  # 11 emb rows per sample

# Chunks = gather granularity (blocks per indirect-DMA pair): small keeps the
# SWDGE ring from blocking; tiny first chunks prime the compute pipeline.
CHUNKS = [1, 1, 2, 2, 2, 2, 2, 2, 2]
# Fold groups (indices into CHUNKS): folds/reduce batch several chunks to
# amortize DVE instruction overhead.
GROUPS = [(0, 1, 2), (3, 4), (5, 6), (7, 8)]
assert sum(CHUNKS) == NBLK

_LAST_RESULTS = None  # test harness introspection (exec_time_ns etc.)


def _build_bass(vocab, debug_dots=False):
    import concourse.bass as bass
    import concourse.tile as tile
    from concourse import bacc, mybir

    total_cols = NBLK * W + NBLK * KP1  # 336
    nc = bacc.Bacc(None, target_bir_lowering=False)
    dots_d = (
        nc.declare_dram_parameter(
            "dots", [P, NBLK * KP1], mybir.dt.float32, isOutput=True
        )
        if debug_dots
        else None
    )
    idx_d = nc.declare_dram_parameter(
        "idx", [P, total_cols], mybir.dt.int32, isOutput=False
    )
    ident_d = nc.declare_dram_parameter(
        "ident", [P, P], mybir.dt.bfloat16, isOutput=False
    )
    ctx_w_d = nc.declare_dram_parameter(
        "ctx_w", [vocab, D], mybir.dt.bfloat16, isOutput=False
    )
    emb_w_d = nc.declare_dram_parameter(
        "emb_w", [vocab, D], mybir.dt.bfloat16, isOutput=False
    )
    out_d = nc.declare_dram_parameter("out", [P, 1], mybir.dt.float32, isOutput=True)

    cbo = np.cumsum([0] + CHUNKS).tolist()  # chunk block offsets

    with tile.TileContext(nc) as tc:
        with (
            tc.tile_pool(name="const", bufs=1) as cpool,
            tc.tile_pool(name="work", bufs=2) as wpool,
            tc.tile_pool(name="psum", bufs=3, space="PSUM") as ppool,
        ):
            idx_sb = cpool.tile([P, total_cols], mybir.dt.int32)
            nc.sync.dma_start(out=idx_sb[:], in_=idx_d[:])
            ident_sb = cpool.tile([P, P], mybir.dt.bfloat16)
            nc.sync.dma_start(out=ident_sb[:], in_=ident_d[:])
            all_dots = cpool.tile([P, NBLK * KP1], mybir.dt.float32)
            acc = cpool.tile([P, 1], mybir.dt.float32)

            # Preload Sigmoid/Ln activation tables at t=0 (no deps) so the
            # tail pass doesn't pay two serial ~1.3us table loads.
            dummy = cpool.tile([P, 8], mybir.dt.float32)
            nc.vector.memset(dummy[:], 1.0)
            nc.scalar.activation(
                out=dummy[:], in_=dummy[:], func=mybir.ActivationFunctionType.Sigmoid
            )
            nc.scalar.activation(
                out=dummy[:], in_=dummy[:], func=mybir.ActivationFunctionType.Ln
            )

            # Per-chunk persistent tiles (no recycling -> no false deps).
            c_sbs, Tctxs, Tembs = [], [], []
            for i, nb in enumerate(CHUNKS):
                c_sb = cpool.tile([P, nb * D], mybir.dt.bfloat16, tag=f"c{i}")
                c_sbs.append(c_sb)
                Tctx = cpool.tile([P, W * nb * D], mybir.dt.bfloat16, tag=f"C{i}")
                Tctxs.append(Tctx)
                Temb = cpool.tile([P, nb * KP1 * D], mybir.dt.bfloat16, tag=f"T{i}")
                Tembs.append(Temb)
            # Per-group product tiles; chunk mults write disjoint slices.
            prods = []
            for gi, g in enumerate(GROUPS):
                gnb = sum(CHUNKS[i] for i in g)
                prod = wpool.tile(
                    [P, gnb * KP1 * D], mybir.dt.bfloat16, tag=f"prod{gi}"
                )
                prods.append(prod)

            # All gathers up front; the SWDGE queue drains them back to back.
            # idx columns are laid out in chunk order: ctx cols then emb cols.
            col = 0
            for i, nb in enumerate(CHUNKS):
                ctx_cols = W * nb
                emb_cols = KP1 * nb
                # ctx: col (u, b) = word u of block (cbo[i]+b), slot-major.
                nc.gpsimd.indirect_dma_start(
                    out=Tctxs[i][:],
                    out_offset=None,
                    in_=ctx_w_d[:],
                    in_offset=bass.IndirectOffsetOnAxis(
                        ap=idx_sb[:, col : col + ctx_cols], axis=0
                    ),
                )
                col += ctx_cols
                # emb: col (b, j) = [tgt, noise*10], block-major.
                nc.gpsimd.indirect_dma_start(
                    out=Tembs[i][:],
                    out_offset=None,
                    in_=emb_w_d[:],
                    in_offset=bass.IndirectOffsetOnAxis(
                        ap=idx_sb[:, col : col + emb_cols], axis=0
                    ),
                )
                col += emb_cols

            # Context pooling: per chunk, one explicit ldweights (constant
            # identity) + 10 PSUM-accumulating matmuls with the implicit
            # weight reload suppressed (no PE array flush between them).
            for i, nb in enumerate(CHUNKS):
                c_ps = ppool.tile([P, 2 * D], mybir.dt.float32, tag="cps")
                for u in range(W):
                    mm = nc.tensor.matmul(
                        c_ps[:, : nb * D],
                        lhsT=ident_sb[:],
                        rhs=Tctxs[i][:, u * nb * D : (u + 1) * nb * D],
                        start=(u == 0),
                        stop=(u == W - 1),
                    )
                    if u > 0:
                        # identity already in the PE array from u=0's load;
                        # skip the reload (and the array flush it causes).
                        mm.ins.ldweights = False
                nc.scalar.activation(
                    out=c_sbs[i][:],
                    in_=c_ps[:, : nb * D],
                    func=mybir.ActivationFunctionType.Copy,
                )

            # Dots: multiply per chunk (2x), fold chain + reduce per group.
            for gi, g in enumerate(GROUPS):
                prod = prods[gi]
                gnb = sum(CHUNKS[i] for i in g)
                poff = 0
                for i in g:
                    nb = CHUNKS[i]
                    cview = c_sbs[i][:].rearrange("p (b d) -> p b d", b=nb)
                    nc.vector.tensor_tensor(
                        out=prod[:, poff : poff + nb * KP1 * D],
                        in0=Tembs[i][:],
                        in1=cview.unsqueeze(2).broadcast_to([P, nb, KP1, D]),
                        op=mybir.AluOpType.mult,
                    )
                    poff += nb * KP1 * D
                seg = gnb * KP1
                f64 = wpool.tile([P, seg * 64], mybir.dt.bfloat16, tag="f64")
                pv = prod[:].rearrange("p (s d) -> p s d", d=D)
                nc.vector.tensor_tensor(
                    out=f64[:],
                    in0=pv[:, :, 0:64],
                    in1=pv[:, :, 64:128],
                    op=mybir.AluOpType.add,
                )
                f32 = wpool.tile([P, seg * 32], mybir.dt.bfloat16, tag="f32")
                fv = f64[:].rearrange("p (s d) -> p s d", d=64)
                nc.vector.tensor_tensor(
                    out=f32[:],
                    in0=fv[:, :, 0:32],
                    in1=fv[:, :, 32:64],
                    op=mybir.AluOpType.add,
                )
                f16 = wpool.tile([P, seg * 16], mybir.dt.bfloat16, tag="f16")
                gv = f32[:].rearrange("p (s d) -> p s d", d=32)
                nc.vector.tensor_tensor(
                    out=f16[:],
                    in0=gv[:, :, 0:16],
                    in1=gv[:, :, 16:32],
                    op=mybir.AluOpType.add,
                )
                doff = cbo[g[0]] * KP1
                nc.vector.tensor_reduce(
                    out=all_dots[:, doff : doff + seg],
                    in_=f16[:].rearrange("p (s d) -> p s d", d=16),
                    axis=mybir.AxisListType.X,
                    op=mybir.AluOpType.add,
                )

            # One tail pass: log-sigmoid of all dots (0.1 rescales the ctx sum
            # to a mean); Ln's accum_out emits per-partition sums.
            sig = cpool.tile([P, NBLK * KP1], mybir.dt.float32)
            nc.scalar.activation(
                out=sig[:],
                in_=all_dots[:],
                func=mybir.ActivationFunctionType.Sigmoid,
                scale=1.0 / W,
            )
            ls = cpool.tile([P, NBLK * KP1], mybir.dt.float32)
            nc.scalar.activation(
                out=ls[:],
                in_=sig[:],
                func=mybir.ActivationFunctionType.Ln,
                accum_out=acc[:, 0:1],
            )

            nc.sync.dma_start(out=out_d[:], in_=acc[:])
            if dots_d is not None:
                nc.sync.dma_start(out=dots_d[:], in_=all_dots[:])
    nc.compile()
    return nc


def _pack_indices(context, target, noise):
    """Per-core [P, 336] int32 index matrices in gather layout."""
    ctx_r = np.ascontiguousarray(context, dtype=np.int32).reshape(NCORES, NBLK, P, W)
    tgt_r = np.ascontiguousarray(target, dtype=np.int32).reshape(NCORES, NBLK, P)
    noi_r = np.ascontiguousarray(noise, dtype=np.int32).reshape(NCORES, NBLK, P, K)
    cbo = np.cumsum([0] + CHUNKS).tolist()
    idxs = []
    for n in range(NCORES):
        cols = []
        for i, nb in enumerate(CHUNKS):
            b0 = cbo[i]
            # ctx cols u-major: col (u, b) partition p = context[block b0+b,
            # sample p, word u]
            csg = ctx_r[n, b0 : b0 + nb]  # [nb, P, W]
            cols.append(csg.transpose(2, 0, 1).reshape(W * nb, P).T)
            # emb cols block-major: col (b, j) = [tgt, noise] for sample p
            esg = np.concatenate(
                [tgt_r[n, b0 : b0 + nb, :, None], noi_r[n, b0 : b0 + nb]], axis=2
            )  # [nb, P, 11]
            cols.append(esg.transpose(0, 2, 1).reshape(nb * KP1, P).T)
        idxs.append(np.ascontiguousarray(np.concatenate(cols, axis=1)))
    return idxs


def kernel(context, target, noise, emb_w, ctx_w):
    global _LAST_RESULTS
    import os
    import sys

    for p in ("/root/.axon_site/_ro/trn_rl_repo", "/opt/trn_rl_repo"):
        if p not in sys.path:
            sys.path.insert(0, p)
    import ml_dtypes

    from concourse.bass_utils import run_bass_kernel_spmd

    context = np.asarray(context)
    target = np.asarray(target)
    noise = np.asarray(noise)
    bf16 = ml_dtypes.bfloat16
    emb_w = np.ascontiguousarray(np.asarray(emb_w, dtype=np.float32).astype(bf16))
    ctx_w = np.ascontiguousarray(np.asarray(ctx_w, dtype=np.float32).astype(bf16))

    debug_dots = bool(os.environ.get("KERNEL_DEBUG_DOTS"))
    nc = _build_bass(V, debug_dots=debug_dots)
    idxs = _pack_indices(context, target, noise)
    ident = np.eye(P, dtype=np.float32).astype(bf16)
    in_maps = [
        {"idx": idxs[n], "ident": ident, "ctx_w": ctx_w, "emb_w": emb_w}
        for n in range(NCORES)
    ]
    tmpdir = os.environ.get("KERNEL_TMPDIR") or None
    res = run_bass_kernel_spmd(nc, in_maps, list(range(NCORES)), tmpdir=tmpdir)
    _LAST_RESULTS = res
    if debug_dots:
        _check_dots(res, context, target, noise, emb_w, ctx_w)
    total = sum(
        float(np.sum(np.asarray(r["out"], dtype=np.float64))) for r in res.results
    )
    return np.float32(-total / B)


def _check_dots(res, context, target, noise, emb_w, ctx_w):
    """Compare on-device dot products (pre-sigmoid, x10 scale) vs numpy."""
    ctx_f = np.asarray(ctx_w, dtype=np.float32)
    emb_f = np.asarray(emb_w, dtype=np.float32)
    c_ref = ctx_f[np.asarray(context)].sum(axis=1)  # [B, D] (sum, not mean)
    eidx = np.concatenate(
        [np.asarray(target)[:, None], np.asarray(noise)], axis=1
    )  # [B, 11]
    dots_ref = np.einsum("bjd,bd->bj", emb_f[eidx], c_ref)  # [B, 11]
    worst = 0.0
    for n, r in enumerate(res.results):
        got = np.asarray(r["dots"], dtype=np.float32)  # [P, NBLK*KP1]
        b0g = n * B_LOCAL
        for blk in range(NBLK):
            ref = dots_ref[b0g + blk * P : b0g + (blk + 1) * P]  # [P, 11]
            sl = got[:, blk * KP1 : (blk + 1) * KP1]  # [P, 11]
            err = np.abs(sl - ref) / (np.abs(ref) + 1e-6)
            worst = max(worst, float(err.max()))
    print(f"debug dots: worst rel err vs numpy-fp32 = {worst:.3e}")
